# revision 37
# baseline (speedup 1.0000x reference)
"""Causal self-attention (B=2, S=2048, D=1024, H=16) on 8 trn2 NeuronCores.

Sharding: core c handles batch b = c // 4 and head-group g = c % 4 (4 heads,
256 feature columns).  QKV weights are column-sharded, the output projection
is row-sharded (Megatron style); the host sums the 4 bf16 partial outputs per
batch and adds the (wv_b @ wo_w + wo_b) correction vector.

Device-side layout (per core), bf16 matmul operands, fp32 psum accumulate:
  xT        [128, 8, 2048]   x[b].T, host pre-transposed (d on partitions)
  qT/kT     2 x [128, 2048]  per fs half: rows = local features (2 heads)
  v_all     [128, 16, 260]   per s-tile: 4 heads x (64 v columns + ones col)
  logits^T  psum [128, <=1024] one k-tile pair, exp'd in a single ACT op
  av^T      psum [65, 512]   rows 0-63 = unnormalized out^T, row 64 = denom
  avT       2 x [128, 2048]  normalized attention output, transposed
  out       [2048, 1024]     partial (pre-reduction) output, bf16

Schedule: per q-block, each head runs its k-tile pairs with the AV matmuls
lagging one pair behind the QK/exp pipeline; diagonal (causally-narrowed)
k-tiles go first, column-compacted to their live q range so QK/exp/
affine_select/AV all skip the masked half.  Projection and out-projection
work is split into small units rationed evenly between pair slots (Feeder)
so the PE always has filler while ACT chews on exp.  PSUM: "small" x2 +
"av" x2 + "lg" ([128,1024]) x2x2 = 8 banks.  The custom-ucode ops
(partition_broadcast, reciprocal_approx_fast) are invoked once at kernel
start to hide their ~6us IRAM library load inside the input-DMA bubble.
Input DMAs are few large issues (each dma_start costs ~650ns of Sync issue
time), ordered so the first projection's operands land first.
"""

import os

import ml_dtypes
import numpy as np

import concourse.bass as bass
import concourse.mybir as mybir
import concourse.tile as tile
from concourse import bacc
from concourse.bass_utils import run_bass_kernel_spmd

F32 = mybir.dt.float32
F32R = mybir.dt.float32r
BF16 = mybir.dt.bfloat16
AF = mybir.ActivationFunctionType

B, S, D = 2, 2048, 1024
H, DH = 16, 64          # heads, head depth
G = 4                   # head groups (cores per batch)
HPG = H // G            # heads per group = 4
F = HPG * DH            # local feature columns = 256
KC = D // 128           # contraction chunks = 8
ST = S // 128           # seq tiles of 128 = 16
QB = S // 512           # q blocks of 512 = 4
SCALE = 1.0 / float(np.sqrt(DH))


def _build(allones: bool):
    nc = bacc.Bacc("TRN2", target_bir_lowering=False, debug=False)

    xT_d = nc.dram_tensor("xT", [4, 128, KC, 512], BF16, kind="ExternalInput")
    wqk_d = nc.dram_tensor("wqk", [2, 128, 2, KC, 128], BF16, kind="ExternalInput")
    wv_d = nc.dram_tensor("wv", [128, KC, F], BF16, kind="ExternalInput")
    wo_d = nc.dram_tensor("wo", [128, 2, D], BF16, kind="ExternalInput")
    bqk_d = nc.dram_tensor("bqk", [128, 4], F32, kind="ExternalInput")
    out_d = nc.dram_tensor("out", [S, D], BF16, kind="ExternalOutput")
    dbg = bool(int(os.environ.get("KDBG", "0")))
    if dbg:
        dbg_qT = nc.dram_tensor("dbg_qT", [2, 128, S], BF16, kind="ExternalOutput")
        dbg_kT = nc.dram_tensor("dbg_kT", [2, 128, S], BF16, kind="ExternalOutput")
        dbg_v = nc.dram_tensor("dbg_v", [128, ST, HPG * 65], BF16, kind="ExternalOutput")
        dbg_avT = nc.dram_tensor("dbg_avT", [2, 128, S], BF16, kind="ExternalOutput")
    if not allones:
        pad_d = nc.dram_tensor("pad", [1, S], BF16, kind="ExternalInput")

    with tile.TileContext(nc) as tc:
        with (
            tc.tile_pool(name="singles", bufs=1) as singles,
            tc.tile_pool(name="expp", bufs=8) as expp,
            tc.tile_pool(name="recipp", bufs=6) as recipp,
            tc.tile_pool(name="bcsbp", bufs=6) as bcsbp,
            tc.tile_pool(name="outsbp", bufs=4) as outsbp,
            tc.tile_pool(name="psum", bufs=2, space="PSUM") as psum,
        ):
            xT = singles.tile([128, 4, KC, 512], BF16, tag="xT")
            wqk = singles.tile([128, 2, 2, KC, 128], BF16, tag="wqk")
            wv = singles.tile([128, KC, F], BF16, tag="wv")
            wo = singles.tile([128, 2, D], BF16, tag="wo")
            qT = [singles.tile([128, S], BF16, tag=f"qT{i}", name=f"qT{i}") for i in range(2)]
            kT = [singles.tile([128, S], BF16, tag=f"kT{i}", name=f"kT{i}") for i in range(2)]
            v_all = singles.tile([128, ST, HPG * 65], BF16, tag="v_all")
            avT = [singles.tile([128, S], BF16, tag=f"avT{i}", name=f"avT{i}") for i in range(2)]
            bqk = singles.tile([128, 4], F32, tag="bqk")

            # --- constant / input DMAs: few, large issues (each dma_start
            # costs ~650ns of Sync issue time), ordered so the first
            # projection groups (fs0 weights + x chunk 0) land first and
            # xT chunk 1 lands before q-block 1 starts ---
            nc.sync.dma_start(out=wqk[:, 0, :, 0:4], in_=wqk_d.ap()[0, :, :, 0:4])
            nc.sync.dma_start(out=xT[:, 0, 0:2], in_=xT_d.ap()[0, :, 0:2])
            nc.sync.dma_start(out=wqk[:, 0, :, 4:8], in_=wqk_d.ap()[0, :, :, 4:8])
            nc.sync.dma_start(out=xT[:, 0, 2:4], in_=xT_d.ap()[0, :, 2:4])
            nc.sync.dma_start(out=bqk, in_=bqk_d.ap())
            nc.sync.dma_start(out=xT[:, 0, 4:8], in_=xT_d.ap()[0, :, 4:8])
            nc.sync.dma_start(out=wv, in_=wv_d.ap())
            nc.sync.dma_start(out=xT[:, 1], in_=xT_d.ap()[1])
            nc.sync.dma_start(out=wqk[:, 1], in_=wqk_d.ap()[1])
            nc.sync.dma_start(out=xT[:, 2], in_=xT_d.ap()[2])
            nc.sync.dma_start(out=xT[:, 3], in_=xT_d.ap()[3])
            nc.sync.dma_start(out=wo, in_=wo_d.ap())
            # Build the denominator ones-columns of v_all via the custom
            # gpsimd partition_broadcast and custom DVE reciprocal so their
            # ~6us one-time ucode IRAM loads happen here, hidden inside the
            # startup DMA bubble (a lazy mid-kernel LOAD_LIB stalls all
            # engines for several us).
            ones16 = singles.tile([1, 16], F32, tag="ones16")
            ones16r = singles.tile([1, 16], F32, tag="ones16r")
            ones16b = singles.tile([1, 16], BF16, tag="ones16b")
            nc.vector.memset(ones16, 1.0)
            nc.vector.reciprocal_approx_fast(ones16r, ones16)
            nc.vector.tensor_copy(ones16b, ones16r)
            for h in range(HPG):
                nc.gpsimd.partition_broadcast(
                    v_all[:, :, h * 65 + 64 : h * 65 + 65], ones16b
                )
            if not allones:
                pad_sb = singles.tile([1, S], BF16, tag="pad")
                ones512 = singles.tile([1, 512], BF16, tag="ones512")
                nc.sync.dma_start(out=pad_sb, in_=pad_d.ap())
                nc.vector.memset(ones512, 1.0)

            def qk_unit(sch, fs, which):
                wi = 0 if which == "q" else 1
                dst = qT if which == "q" else kT
                ps = psum.tile([128, 512], F32, tag="small", bufs=2, name="ps_qk")
                for kc in range(KC):
                    nc.tensor.matmul(
                        ps,
                        lhsT=wqk[:, fs, wi, kc, :],
                        rhs=xT[:, sch, kc, :],
                        start=(kc == 0),
                        stop=(kc == KC - 1),
                    )
                nc.vector.tensor_scalar_add(
                    dst[fs][:, bass.ds(sch * 512, 512)],
                    ps,
                    bqk[:, 2 * wi + fs : 2 * wi + fs + 1],
                )

            def v_unit(st):
                vps = psum.tile([128, F], F32, tag="small", bufs=2, name="ps_v")
                for kc in range(KC):
                    nc.tensor.matmul(
                        vps,
                        lhsT=xT[:, st // 4, kc, bass.ds((st % 4) * 128, 128)],
                        rhs=wv[:, kc, :],
                        start=(kc == 0),
                        stop=(kc == KC - 1),
                    )
                for h in range(HPG):
                    nc.vector.tensor_copy(
                        v_all[:, st, h * 65 : h * 65 + 64],
                        vps[:, h * 64 : (h + 1) * 64],
                    )

            ob_tiles: dict = {}

            def oproj_unit(st, eh, scalar_cast=False, split_dma=False):
                if eh == 0:
                    ob_tiles[st] = outsbp.tile([128, D], BF16, tag="ob", name="ob")
                ob = ob_tiles[st]
                op = psum.tile([128, 512], F32, tag="small", bufs=2, name="ps_op")
                for fs in range(2):
                    nc.tensor.matmul(
                        op,
                        lhsT=avT[fs][:, bass.ds(st * 128, 128)],
                        rhs=wo[:, fs, bass.ds(eh * 512, 512)],
                        start=(fs == 0),
                        stop=(fs == 1),
                    )
                if scalar_cast:
                    # the kernel tail has an idle ACT engine; DVE casts are
                    # the pacer there
                    nc.scalar.activation(
                        ob[:, bass.ds(eh * 512, 512)], op, AF.Copy
                    )
                else:
                    nc.vector.tensor_copy(ob[:, bass.ds(eh * 512, 512)], op)
                if split_dma:
                    # tail: DMA each half as soon as its cast lands so the
                    # final drain isn't waiting on one big transfer
                    nc.sync.dma_start(
                        out=out_d.ap()[bass.ds(st * 128, 128), bass.ds(eh * 512, 512)],
                        in_=ob[:, bass.ds(eh * 512, 512)],
                    )
                    if eh == 1:
                        del ob_tiles[st]
                elif eh == 1:
                    nc.sync.dma_start(
                        out=out_d.ap()[bass.ds(st * 128, 128)], in_=ob
                    )
                    del ob_tiles[st]

            class Feeder:
                """Rations filler units (closures) across attention pair slots."""

                def __init__(self, units, slots):
                    self.units = list(units)
                    self.per = len(self.units) / max(1, slots)
                    self.credit = 0.0
                    self.i = 0

                def feed(self):
                    self.credit += self.per
                    while self.credit >= 1.0 and self.i < len(self.units):
                        self.units[self.i]()
                        self.i += 1
                        self.credit -= 1.0

                def drain(self):
                    while self.i < len(self.units):
                        self.units[self.i]()
                        self.i += 1

            def head_plan(qb):
                # diagonal (causally-narrowed) kt tiles first, then full tiles;
                # the first emitted kt (4*qb, full width) carries start=True
                nkt = 4 * qb + 4
                order = list(range(4 * qb, nkt)) + list(range(0, 4 * qb))
                pairs = [order[i : i + 2] for i in range(0, nkt, 2)]
                return pairs, 4 * qb, (4 * qb - 1) % nkt

            def emit_qk_pair(qb, h, kts):
                fs, hh = h // 2, h % 2
                hsl = bass.ds(hh * 64, 64)
                offs = [max(0, (kt - 4 * qb) * 128) for kt in kts]
                widths = [512 - o for o in offs]
                lg = psum.tile([128, 1024], F32, tag="lg", name="ps_lg")
                c = 0
                cols = []
                for kt, qoff, w in zip(kts, offs, widths):
                    cols.append(c)
                    if not allones:
                        nc.tensor.matmul(
                            lg[:, bass.ds(c, w)],
                            lhsT=pad_sb[:, bass.ds(kt * 128, 128)],
                            rhs=ones512[:, :w],
                            start=True,
                            stop=False,
                        )
                    nc.tensor.matmul(
                        lg[:, bass.ds(c, w)],
                        lhsT=kT[fs][hsl, bass.ds(kt * 128, 128)],
                        rhs=qT[fs][hsl, bass.ds(qb * 512 + qoff, w)],
                        start=allones,
                        stop=True,
                    )
                    c += w
                ex = expp.tile([128, 1024], BF16, tag="ex", name="ex")
                nc.scalar.activation(ex[:, :c], lg[:, :c], AF.Exp, scale=SCALE)
                for kt, qoff, w, cc in zip(kts, offs, widths, cols):
                    if kt >= 4 * qb:  # diagonal tile: causal mask
                        nc.gpsimd.affine_select(
                            out=ex[:, bass.ds(cc, w)],
                            in_=ex[:, bass.ds(cc, w)],
                            compare_op=mybir.AluOpType.is_ge,
                            fill=0.0,
                            base=0,
                            channel_multiplier=-1,
                            pattern=[[1, w]],
                        )
                return ex, list(zip(kts, offs, widths, cols))

            def emit_av_pair(qb, h, av, meta, start_kt, stop_kt):
                ex, items = meta
                for kt, qoff, w, cc in items:
                    nc.tensor.matmul(
                        av[:, bass.ds(qoff, w)],
                        lhsT=v_all[:, kt, h * 65 : (h + 1) * 65],
                        rhs=ex[:, bass.ds(cc, w)],
                        start=(kt == start_kt),
                        stop=(kt == stop_kt),
                    )

            def emit_head(qb, h, feeder):
                fs, hh = h // 2, h % 2
                hsl = bass.ds(hh * 64, 64)
                av = psum.tile([65, 512], F32, tag="av", bufs=2, name="ps_av")
                pairs, start_kt, stop_kt = head_plan(qb)
                # AV matmuls lag their QK/exp by two pairs so the PE never
                # waits on the ACT exp + gpsimd mask chain
                pending: list = []
                for kts in pairs:
                    meta = emit_qk_pair(qb, h, kts)
                    feeder.feed()
                    pending.append(meta)
                    if len(pending) > 2:
                        emit_av_pair(qb, h, av, pending.pop(0), start_kt, stop_kt)
                for meta in pending:
                    emit_av_pair(qb, h, av, meta, start_kt, stop_kt)
                # normalize: row 64 of av holds the softmax denominator
                den = recipp.tile([1, 512], F32, tag="den", name="den")
                nc.vector.tensor_copy(den, av[64:65, :])
                rf = recipp.tile([1, 512], F32, tag="rf", name="rf")
                nc.vector.reciprocal_approx_fast(rf, den)
                bcast = bcsbp.tile([64, 512], F32, tag="bcast", name="bcast")
                nc.gpsimd.partition_broadcast(bcast, rf)
                nc.vector.tensor_mul(
                    avT[fs][hsl, bass.ds(qb * 512, 512)], av[0:64, :], bcast
                )

            with nc.named_scope("proj0"):
                qk_unit(0, 0, "q")
                qk_unit(0, 0, "k")
                for st in range(4):
                    v_unit(st)

            for qb in range(QB):
                nsch = qb + 1
                proj_units = []
                if qb == 0:
                    proj_units += [
                        lambda: qk_unit(0, 1, "q"),
                        lambda: qk_unit(0, 1, "k"),
                    ]
                if nsch < QB:
                    proj_units += [
                        lambda: qk_unit(nsch, 0, "q"),
                        lambda: qk_unit(nsch, 0, "k"),
                    ]
                    proj_units += [
                        (lambda st=st: v_unit(st))
                        for st in range(4 * nsch, 4 * nsch + 4)
                    ]
                    proj_units += [
                        lambda: qk_unit(nsch, 1, "q"),
                        lambda: qk_unit(nsch, 1, "k"),
                    ]
                # oproj of q-block qb-1 fills qb's pair slots; qb3 (which has
                # no projection fillers) additionally takes oproj of qb1
                oproj_qbs = {1: [], 2: [0], 3: [1, 2]}[qb] if qb > 0 else []
                oproj_units = [
                    (lambda st=st, eh=eh: oproj_unit(st, eh))
                    for oqb in oproj_qbs
                    for st in range(4 * oqb, 4 * oqb + 4)
                    for eh in range(2)
                ]
                # interleave the two pools so DVE casts spread out
                units = []
                pi = oi = 0
                while pi < len(proj_units) or oi < len(oproj_units):
                    if pi < len(proj_units):
                        units.append(proj_units[pi])
                        pi += 1
                    if oi < len(oproj_units):
                        units.append(oproj_units[oi])
                        oi += 1
                # qb3 keeps a couple of units in reserve so the PE has work
                # during the final head's normalization chain
                slots = HPG * (2 * qb + 2)
                feeder = Feeder(units, slots=(slots * 3) // 2 if qb == QB - 1 else slots)
                with nc.named_scope(f"qb{qb}"):
                    for h in range(HPG):
                        emit_head(qb, h, feeder)
                    feeder.drain()
            with nc.named_scope("oproj_last"):
                for st in range(4 * (QB - 1), 4 * (QB - 1) + 4):
                    for eh in range(2):
                        oproj_unit(st, eh, scalar_cast=(eh == 0), split_dma=True)

            if dbg:
                for i in range(2):
                    nc.sync.dma_start(out=dbg_qT.ap()[i], in_=qT[i])
                    nc.sync.dma_start(out=dbg_kT.ap()[i], in_=kT[i])
                    nc.sync.dma_start(out=dbg_avT.ap()[i], in_=avT[i])
                nc.sync.dma_start(out=dbg_v.ap(), in_=v_all)

    nc.compile()
    return nc


_CACHE: dict = {}


def kernel(
    x,
    padding_mask,
    wq_w,
    wq_b,
    wk_w,
    wk_b,
    wv_w,
    wv_b,
    wo_w,
    wo_b,
    **trace_kwargs,
):
    x = np.asarray(x, dtype=np.float32)
    padding_mask = np.asarray(padding_mask, dtype=np.float32)
    wq_w = np.asarray(wq_w, dtype=np.float32)
    wk_w = np.asarray(wk_w, dtype=np.float32)
    wv_w = np.asarray(wv_w, dtype=np.float32)
    wo_w = np.asarray(wo_w, dtype=np.float32)
    wq_b = np.asarray(wq_b, dtype=np.float32)
    wk_b = np.asarray(wk_b, dtype=np.float32)
    wv_b = np.asarray(wv_b, dtype=np.float32)
    wo_b = np.asarray(wo_b, dtype=np.float32)

    allones = bool(np.all(padding_mask == 1.0))
    if allones not in _CACHE:
        _CACHE[allones] = _build(allones)
    nc = _CACHE[allones]

    bf = ml_dtypes.bfloat16
    in_maps = []
    for c in range(8):
        b, g = c // 4, c % 4
        fsl = slice(g * F, (g + 1) * F)
        xTb = x[b].T.astype(bf)  # (1024, 2048)
        m = {
            # [4 sch, 128 p, KC, 512]: xT[d, s] with d = kc*128 + p
            "xT": np.ascontiguousarray(
                xTb.reshape(KC, 128, 4, 512).transpose(2, 1, 0, 3)
            ),
            # [fs, 128 p, which(q/k), KC, 128]: column block fs of the
            # local F columns, q and k packed in one tensor per fs
            "wqk": np.ascontiguousarray(
                np.stack(
                    [
                        wq_w[:, fsl].astype(bf).reshape(KC, 128, 2, 128),
                        wk_w[:, fsl].astype(bf).reshape(KC, 128, 2, 128),
                    ],
                    axis=0,
                ).transpose(3, 2, 0, 1, 4)
            ),
            "wv": np.ascontiguousarray(
                wv_w[:, fsl].astype(bf).reshape(KC, 128, F).transpose(1, 0, 2)
            ),
            "wo": np.ascontiguousarray(
                wo_w[fsl, :].astype(bf).reshape(2, 128, D).transpose(1, 0, 2)
            ),
            # [128 p, 4]: cols 0-1 = q bias (fs0, fs1), 2-3 = k bias
            "bqk": np.ascontiguousarray(
                np.stack(
                    [
                        wq_b[fsl].reshape(2, 128)[0],
                        wq_b[fsl].reshape(2, 128)[1],
                        wk_b[fsl].reshape(2, 128)[0],
                        wk_b[fsl].reshape(2, 128)[1],
                    ],
                    axis=1,
                ).astype(np.float32)
            ),
        }
        if not allones:
            m["pad"] = ((padding_mask[b] - 1.0) * 8e9).reshape(1, S).astype(bf)
        in_maps.append(m)

    def _run_and_reduce(**kw):
        res = run_bass_kernel_spmd(nc, in_maps, core_ids=list(range(8)), **kw)
        correction = (wv_b @ wo_w + wo_b).astype(np.float32)
        out = np.empty((B, S, D), dtype=np.float32)
        for b in range(B):
            acc = res.results[4 * b]["out"].astype(np.float32)
            for g in range(1, G):
                acc += res.results[4 * b + g]["out"].astype(np.float32)
            out[b] = acc + correction
        return res, out

    res, out = _run_and_reduce(**trace_kwargs)
    if not np.isfinite(out).all():
        # rare transient device-side glitch: rerun once
        res, out = _run_and_reduce()
    kernel._last_results = res
    return out

